# revision 45
# baseline (speedup 1.0000x reference)
"""Trainium2 Bass kernel for nn_Attention_146028888114.

Full attention block: LN -> QKV -> per-head QK-LN -> RoPE -> SDPA -> out-proj.
B=2, S=2048, D=1024, H=16, HD=64 (fp16 compute, f32 PSUM accumulation).

Sharding: DP over batch (2 groups of 4 cores) x TP over heads (4 heads/core).
Each core computes a partial out-projection (its 4 heads' contribution); the
host sums the 4 partials per batch (the unshard/reduce step).

Structure (v2):
- Host supplies x twice in fp16: token-major (LN stats) and pre-transposed
  per-tile (matmul lhsT) -- no PE transposes of x.
- Input LN folded into the QKV matmul: qkv = (x - mu) @ W via a rank-1
  correction matmul (lhsT = -mu row, rhs = column-sums of W). The 1/std
  factor cancels in the per-head QK-LN for q/k and is applied to v as a
  per-partition ACT scale.
- Per-head QK-LN stats from one grouped bn_stats (even/odd 6-stat form).
- Phase 2: head-pair outermost; scores -> exp -> attnV software-pipelined so
  ACT (exp) stays saturated; softmax normalize + out-proj for chunk j are
  deferred into chunk j+1's loop.
"""

import sys

sys.path.insert(0, "/opt/trn_rl_repo")

import numpy as np

import concourse.bass as bass
import concourse.tile as tile
from concourse import bacc, mybir
from concourse.alu_op_type import AluOpType
from concourse.bass_utils import run_bass_kernel_spmd
from concourse.masks import make_identity

B, S, D = 2, 2048, 1024
H, HD = 16, 64
EPS = 1e-6
ROPE_BASE = 10000.0

NCORES = 8
GROUPS = 4            # cores per batch group (TP degree)
HLOC = H // GROUPS    # heads per core
P = 128
ST = S // P           # 16 s-tiles
KC = D // P           # 8 k-chunks of the QKV contraction
NQKV = 3 * HLOC * HD  # 768 qkv columns per core
NQK = 2 * HLOC * HD   # 512 q+k columns
SQW = 512             # sq chunk width
SQC = S // SQW        # 4 sq chunks
# v cols + ones col at 64 (denominator trick); padded to a 32 multiple:
# matmul silently zeroes output rows past M=64 when M isn't 32-aligned.
VW = 96

DT = mybir.dt
F32 = DT.float32
F16 = DT.float16
AF = mybir.ActivationFunctionType
F16NP = np.float16


def build_program(use_bias_qkv: bool, use_bias_out: bool, stop: int = 5,
                  debug_dump: bool = False, repeat: int = 1):
    if use_bias_qkv:
        raise NotImplementedError("non-zero qkv/ln bias not supported by v2 kernel")
    nc = bacc.Bacc("TRN2", target_bir_lowering=False, debug=False, num_devices=NCORES)

    x_d = nc.dram_tensor("x", [S, D], F16, kind="ExternalInput")
    xT_d = nc.dram_tensor("xT", [P, ST * KC * P], F16, kind="ExternalInput")
    wqkv_d = nc.dram_tensor("wqkv", [P, KC * NQKV], F16, kind="ExternalInput")
    wout_d = nc.dram_tensor("wout", [P, 2 * D], F16, kind="ExternalInput")
    bout_d = nc.dram_tensor("bout", [1, D], F16, kind="ExternalInput")
    tab_d = {nm: nc.dram_tensor(nm, [P, ST * HD], F16, kind="ExternalInput")
             for nm in ("cq", "sq", "ck", "sk")}
    out_d = nc.dram_tensor("out", [S, D], F32, kind="ExternalOutput")
    rec_d = nc.dram_tensor("recscratch", [2 * SQC, 2 * SQW], F16)
    if debug_dump:
        dbg_qT = nc.dram_tensor("dbg_qT", [P, 2 * S], F16, kind="ExternalOutput")
        dbg_kT = nc.dram_tensor("dbg_kT", [P, 2 * S], F16, kind="ExternalOutput")
        dbg_v = nc.dram_tensor("dbg_v", [P, ST * HLOC * VW], F16, kind="ExternalOutput")
        dbg_an = nc.dram_tensor("dbg_an", [P, 2 * S], F16, kind="ExternalOutput")
        dbg_qkn = nc.dram_tensor("dbg_qkn", [S, 8 * HD], F16, kind="ExternalOutput")

    with tile.TileContext(nc) as tc:
        with tc.tile_pool(name="const", bufs=1) as cpool, \
             tc.tile_pool(name="data", bufs=1) as dpool:
            # --- constants ---
            ident = cpool.tile([P, P], F32)
            make_identity(nc, ident[:])
            ident_h = cpool.tile([P, P], F16)
            nc.vector.tensor_copy(ident_h[:], ident[:])

            ones1h = cpool.tile([1, P], F16)
            nc.vector.memset(ones1h[:], 1.0)

            eps_t = cpool.tile([P, 1], F32)
            nc.vector.memset(eps_t[:], EPS)
            eps64_t = cpool.tile([P, 1], F32)
            nc.vector.memset(eps64_t[:], float(HD) * EPS)

            # --- persistent data tiles ---
            qT = dpool.tile([P, 2, S], F16, tag="qT")   # [pair-features, hp, s]
            kT = dpool.tile([P, 2, S], F16, tag="kT")
            v_all = dpool.tile([P, ST, HLOC, VW], F16, tag="v")
            attnN = dpool.tile([P, 2, S], F16, tag="attnN")

            # --- weights / tables: host supplies fp16 pre-tiled, direct DMA ---
            w_r = cpool.tile([P, KC, NQKV], F16)
            nc.sync.dma_start(w_r[:].rearrange("p a b -> p (a b)"), wqkv_d[:])
            wout_r = cpool.tile([P, 2, D], F16)
            nc.sync.dma_start(wout_r[:].rearrange("p a b -> p (a b)"), wout_d[:])
            if use_bias_out:
                bo_r = cpool.tile([1, D], F16)
                nc.sync.dma_start(bo_r[:], bout_d[:])
            tabs = {}
            for nm, dram in tab_d.items():
                tt = cpool.tile([P, ST, HD], F16, tag=f"tab_{nm}")
                nc.sync.dma_start(tt[:].rearrange("p a b -> p (a b)"), dram[:])
                tabs[nm] = tt

            # ones columns of v (denominator trick): one ACT op
            onescol_f = cpool.tile([P, 1], F32)
            nc.vector.memset(onescol_f[:], 1.0)
            nc.scalar.activation(
                v_all[:, :, :, HD:VW],
                onescol_f[:, :, None, None].to_broadcast((P, ST, HLOC, VW - HD)),
                AF.Copy,
            )

            for _rep in range(repeat):
              # ------------- Phase 1: LN-folded QKV + QK-LN + RoPE ------------
              if stop >= 2:
                with tc.tile_pool(name="xp", bufs=4) as xp, \
                   tc.tile_pool(name="xtp", bufs=4) as xtp, \
                   tc.tile_pool(name="ph1", bufs=4) as ph1, \
                   tc.tile_pool(name="ph1s", bufs=8) as ph1s, \
                   tc.tile_pool(name="tq_ps", bufs=2, space="PSUM") as tqp, \
                   tc.tile_pool(name="qkv_ps", bufs=3, space="PSUM") as qkvp:
                  rope_pending = []  # deferred rope-transposes (one-tile lag)
                  for t in range(ST):
                      x_t = xp.tile([P, 2, 512], F16, tag="x")
                      nc.sync.dma_start(
                          x_t[:].rearrange("p a b -> p (a b)"),
                          x_d[t * P:(t + 1) * P, :])
                      xT_t = xtp.tile([P, KC, P], F16, tag="xT")
                      nc.sync.dma_start(
                          xT_t[:].rearrange("p a b -> p (a b)"),
                          xT_d[:, t * KC * P:(t + 1) * KC * P])

                      # input LN stats (grouped bn_stats + aggregate)
                      st1 = ph1s.tile([P, 2, 6], F32, tag="st1")
                      nc.vector.bn_stats(st1[:, 0, :], x_t[:, 0, :])
                      nc.vector.bn_stats(st1[:, 1, :], x_t[:, 1, :])
                      mv = ph1s.tile([P, 2], F32, tag="mv")
                      nc.vector.bn_aggr(mv[:], st1[:])
                      rstd = ph1s.tile([P, 1], F32, tag="rstd")
                      nc.scalar.activation(rstd[:], mv[:, 1:2], AF.Sqrt, bias=eps_t[:])
                      nc.vector.reciprocal(rstd[:], rstd[:])

                      # QKV projection (both LN means pre-folded into weights)
                      qkv_ps = qkvp.tile([P, NQKV], F32, tag="qkv")
                      nsl = [(0, 512), (512, NQKV)]
                      for k in range(KC):
                          for lo, hi in nsl:
                              nc.tensor.matmul(
                                  qkv_ps[:, lo:hi], lhsT=xT_t[:, k, :],
                                  rhs=w_r[:, k, lo:hi],
                                  start=(k == 0), stop=(k == KC - 1),
                              )

                      # previous tile's rope transposes after this tile's QKV:
                      # by then their (elementwise-produced) inputs are ready,
                      # so PE never waits on the LN/rope chain
                      for fn in rope_pending:
                          fn()
                      rope_pending = []

                      # v with 1/std scale (per-partition ACT scale)
                      nc.scalar.activation(
                          v_all[:, t, :, 0:HD],
                          qkv_ps[:, NQK:NQKV].rearrange("p (h d) -> p h d", h=HLOC),
                          AF.Copy, scale=rstd[:],
                      )

                      # per-head QK LN: qk is already head-centered (mean folded
                      # into weights), so only the variance is needed:
                      # rstd = 1/sqrt(sumsq/64 + eps) = 8/sqrt(sumsq + 64*eps)
                      qk_g = qkv_ps[:, 0:NQK].rearrange("p (g d) -> p g d", g=8)
                      sq = ph1.tile([P, 8, HD], F16, tag="sq")
                      nc.scalar.square(
                          sq[:].rearrange("p g d -> p (g d)"), qkv_ps[:, 0:NQK])
                      sumsq = ph1s.tile([P, 8], F32, tag="sumsq")
                      nc.vector.reduce_sum(sumsq[:], sq[:], axis=mybir.AxisListType.X)
                      s8 = ph1s.tile([P, 8], F32, tag="s8")
                      nc.scalar.activation(s8[:], sumsq[:], AF.Sqrt, bias=eps64_t[:])
                      r8 = ph1s.tile([P, 8], F32, tag="r8")
                      nc.vector.reciprocal(r8[:], s8[:])
                      r88 = ph1s.tile([P, 8], F32, tag="r88")
                      nc.vector.tensor_scalar_mul(r88[:], r8[:], 8.0)
                      qkn = ph1.tile([P, 8, HD], F16, tag="qkn")
                      nc.vector.tensor_tensor(
                          qkn[:], qk_g, r88[:, :, None].to_broadcast((P, 8, HD)),
                          op=AluOpType.mult)
                      if debug_dump:
                          nc.sync.dma_start(
                              dbg_qkn[t * P:(t + 1) * P, :],
                              qkn[:].rearrange("p a b -> p (a b)"))

                      # RoPE (q on DVE; k split DVE/Pool) + PE pair-transposes
                      for gsl, ctab, stab, dstT, eng2, eng34 in (
                              (slice(0, HLOC), "cq", "sq", qT, nc.vector, nc.vector),
                              (slice(HLOC, 8), "ck", "sk", kT, nc.gpsimd, nc.gpsimd)):
                          n_t = qkn[:, gsl, :]
                          ct = tabs[ctab][:, t, None, :].to_broadcast((P, HLOC, HD))
                          s_lo = tabs[stab][:, t, None, 0:32].to_broadcast((P, HLOC, 32))
                          s_hi = tabs[stab][:, t, None, 32:64].to_broadcast((P, HLOC, 32))
                          r2 = ph1.tile([P, HLOC, HD], F16, tag="r2")
                          eng2.tensor_tensor(
                              r2[:, :, 0:32], n_t[:, :, 32:64], s_lo, op=AluOpType.mult)
                          eng2.tensor_tensor(
                              r2[:, :, 32:64], n_t[:, :, 0:32], s_hi, op=AluOpType.mult)
                          r3 = ph1.tile([P, HLOC, HD], F16, tag="r3")
                          eng34.tensor_tensor(r3[:], n_t[:], ct, op=AluOpType.mult)
                          r4 = ph1.tile([P, HLOC, HD], F16, tag="r4")
                          eng34.tensor_tensor(r4[:], r3[:], r2[:], op=AluOpType.add)

                          # tq = T(r4 pair); deferred one tile so PE never
                          # waits on the LN/rope chain of the current tile
                          def make_ropet(r4=r4, dstT=dstT, t=t):
                              def _ropet():
                                  tq = tqp.tile([P, 2, P], F16, tag="tq",
                                                name="tq")
                                  for hp in range(2):
                                      nc.tensor.matmul(
                                          tq[:, hp, :],
                                          lhsT=r4[:, 2 * hp:2 * hp + 2, :].rearrange("p h d -> p (h d)"),
                                          rhs=ident_h[:],
                                          is_transpose=True, start=True, stop=True,
                                      )
                                  nc.scalar.activation(
                                      dstT[:, :, t * P:(t + 1) * P], tq[:], AF.Copy)
                              return _ropet
                          rope_pending.append(make_ropet())
                  for fn in rope_pending:
                      fn()
                  rope_pending = []

              if debug_dump:
                  with tc.tile_pool(name="dbgp", bufs=2) as dbgp:
                      for (dst, srcT) in ((dbg_qT, qT), (dbg_kT, kT)):
                          nc.sync.dma_start(dst[:], srcT[:].rearrange("p a b -> p (a b)"))
                      nc.sync.dma_start(
                          dbg_v[:], v_all[:].rearrange("p a b c -> p (a b c)"))

              # ------------- Phase 2+3: attention + out-projection ------------
              if stop >= 3:
                with tc.tile_pool(name="st_ps", bufs=2, space="PSUM") as stp, \
                   tc.tile_pool(name="attn_ps", bufs=1, space="PSUM") as atp, \
                   tc.tile_pool(name="out_ps", bufs=2, space="PSUM") as outp, \
                   tc.tile_pool(name="pt", bufs=6) as ptp, \
                   tc.tile_pool(name="ph3", bufs=2) as ph3, \
                   tc.tile_pool(name="ob", bufs=3) as obp:

                  def make_normalize(hp, j, asb, bc):
                      def _norm():
                          for h2 in range(2):
                              nc.gpsimd.tensor_tensor(
                                  attnN[h2 * HD:(h2 + 1) * HD, hp,
                                        j * SQW:(j + 1) * SQW],
                                  asb[:, h2, :], bc[:, h2, :], op=AluOpType.mult)
                      return _norm

                  # out-proj of one 128-row x 512-col block of chunk j, split
                  # into two single-matmul pieces so the interleave never puts
                  # more than ~213ns of extra PE work into one i-iteration
                  def make_outproj(j, m, lo_i):
                      row = j * SQW + m * P
                      lo = lo_i * 512
                      state = {}

                      def _op1():
                          o_ps = outp.tile([P, 512], F32, tag="out", name="o_ps")
                          state["o_ps"] = o_ps
                          first = True
                          if use_bias_out:
                              nc.tensor.matmul(
                                  o_ps[:], lhsT=ones1h[:], rhs=bo_r[:, lo:lo + 512],
                                  start=True, stop=False)
                              first = False
                          nc.tensor.matmul(
                              o_ps[:], lhsT=attnN[:, 0, row:row + P],
                              rhs=wout_r[:, 0, lo:lo + 512],
                              start=first, stop=False)

                      def _op2():
                          o_ps = state["o_ps"]
                          nc.tensor.matmul(
                              o_ps[:], lhsT=attnN[:, 1, row:row + P],
                              rhs=wout_r[:, 1, lo:lo + 512],
                              start=False, stop=True)
                          ob = obp.tile([P, 512], F32, tag="ob", name="ob")
                          if j == SQC - 1:
                              # final chunk drains with ACT idle; DVE is busy
                              # with this chunk's recips/copies
                              nc.scalar.copy(ob[:], o_ps[:])
                          else:
                              nc.vector.tensor_copy(ob[:], o_ps[:])
                          nc.sync.dma_start(out_d[row:row + P, lo:lo + 512], ob[:])

                      return [_op1, _op2]

                  # leftover phase-1 rope transposes flush early in phase 2
                  # (their consumers -- scores at high i -- are far away)
                  pending = list(rope_pending)
                  rope_pending = []

                  chunks = [(hp, j) for hp in range(2) for j in range(SQC)]
                  accs = {}
                  lag_q = []   # (ci, i, pT) whose attnV is deferred 2 slots
                  LAG = 2

                  def emit_attnv(ci, i, pT):
                      hp, j = chunks[ci]
                      for h2 in range(2):
                          nc.tensor.matmul(
                              accs[ci][h2][:],
                              lhsT=v_all[:, i, 2 * hp + h2, :],
                              rhs=pT[:, h2, :],
                              start=(i == 0), stop=(i == ST - 1))

                  def emit_tail(ci):
                      hp, j = chunks[ci]
                      acc = accs.pop(ci)
                      last = (ci == len(chunks) - 1)
                      asb = ph3.tile([HD, 2, SQW], F16, tag="asb", name="asb")
                      rec = ph3.tile([1, 2, SQW], F16, tag="rec", name="rec")
                      with nc.allow_low_precision(reason="fp16 softmax recip"):
                          for h2 in range(2):
                              nc.vector.reciprocal(
                                  rec[:, h2, :], acc[h2][HD:HD + 1, :])
                              nc.vector.tensor_copy(asb[:, h2, :], acc[h2][0:HD, :])
                      ridx = hp * SQC + j
                      nc.sync.dma_start(
                          rec_d[ridx:ridx + 1, :],
                          rec[:].rearrange("p a b -> p (a b)"))
                      bc = ph3.tile([HD, 2, SQW], F16, tag="bc", name="bc")
                      nc.sync.dma_start(
                          bc[:].rearrange("p a b -> p (a b)"),
                          rec_d[ridx:ridx + 1, :].to_broadcast((HD, 2 * SQW)))
                      if last and stop >= 5:
                          # drain: column-split normalize so each out-proj row
                          # block starts as soon as its 128 query columns are
                          # normalized
                          for m in range(SQW // P):
                              cl = slice(m * P, (m + 1) * P)
                              for h2 in range(2):
                                  nc.gpsimd.tensor_tensor(
                                      attnN[h2 * HD:(h2 + 1) * HD, hp,
                                            j * SQW + m * P:j * SQW + (m + 1) * P],
                                      asb[:, h2, cl], bc[:, h2, cl],
                                      op=AluOpType.mult)
                              for lo_i in range(2):
                                  for fn in make_outproj(j, m, lo_i):
                                      fn()
                      else:
                          pending.append(make_normalize(hp, j, asb, bc))
                          if hp == 1 and stop >= 5:
                              for m in range(SQW // P):
                                  for lo_i in range(2):
                                      pending.extend(make_outproj(j, m, lo_i))

                  for ci, (hp, j) in enumerate(chunks):
                      accs[ci] = [atp.tile([VW, SQW], F32, tag=f"acc{h2}",
                                           name=f"acc{h2}")
                                  for h2 in range(2)]
                      for i in range(ST):
                          sT = stp.tile([P, 2, SQW], F32, tag="st")
                          for h2 in range(2):
                              fl = h2 * HD
                              nc.tensor.matmul(
                                  sT[:, h2, :],
                                  lhsT=kT[fl:fl + HD, hp, i * P:(i + 1) * P],
                                  rhs=qT[fl:fl + HD, hp, j * SQW:(j + 1) * SQW],
                                  start=True, stop=True)
                          pT = ptp.tile([P, 2, SQW], F16, tag="pt")
                          nc.scalar.activation(pT[:], sT[:], AF.Exp, scale=0.125)
                          if pending and (ci > 0 or i >= 1):
                              pending.pop(0)()
                              if len(pending) > 16 - i:
                                  pending.pop(0)()
                          lag_q.append((ci, i, pT))
                          if len(lag_q) > LAG:
                              aci, ai, apT = lag_q.pop(0)
                              emit_attnv(aci, ai, apT)
                              if ai == ST - 1:
                                  emit_tail(aci)
                  while lag_q:
                      aci, ai, apT = lag_q.pop(0)
                      emit_attnv(aci, ai, apT)
                      if ai == ST - 1:
                          for fn in pending:
                              fn()
                          pending = []
                          emit_tail(aci)

              if debug_dump:
                  with tc.tile_pool(name="dbgp2", bufs=1) as dbgp2:
                      nc.sync.dma_start(
                          dbg_an[:], attnN[:].rearrange("p a b -> p (a b)"))

    nc.compile()
    return nc


_PROGRAM_CACHE = {}


def _get_program(use_bias_qkv, use_bias_out):
    key = (use_bias_qkv, use_bias_out)
    if key not in _PROGRAM_CACHE:
        _PROGRAM_CACHE[key] = build_program(*key)
    return _PROGRAM_CACHE[key]


def _rope_tables(q_scale, k_scale):
    inv_freq = 1.0 / ROPE_BASE ** (np.arange(0, HD, 2, dtype=np.float32) / HD)
    t = np.arange(S, dtype=np.float32)
    freqs = np.einsum("i,j->ij", t, inv_freq)
    emb = np.concatenate((freqs, freqs), axis=-1)          # [S, HD]
    cos = np.cos(emb).astype(np.float32)
    sin = np.sin(emb).astype(np.float32)
    sgnsin = sin.copy()
    sgnsin[:, 0:HD // 2] *= -1.0
    swap = lambda v: np.concatenate((v[HD // 2:], v[:HD // 2]))
    tabs = {}
    for nm, sc in (("q", q_scale), ("k", k_scale)):
        tabs["c" + nm] = np.ascontiguousarray(cos * sc[None, :])
        tabs["s" + nm] = np.ascontiguousarray(sgnsin * swap(sc)[None, :])
    return tabs


def _tile_rows(a):
    """[S-like rows, W] -> [P, (rows/P) * W] partition-tiled layout."""
    r, w = a.shape
    return np.ascontiguousarray(
        a.reshape(r // P, P, w).transpose(1, 0, 2).reshape(P, (r // P) * w))


def make_in_maps(x, w_qkv, b_qkv, w_out, b_out, ln_scale, ln_bias, q_scale, k_scale):
    tabs = _rope_tables(q_scale, k_scale)
    tabs_tiled = {nm: _tile_rows(v).astype(F16NP) for nm, v in tabs.items()}
    wq, wk, wv = w_qkv[:, 0:D], w_qkv[:, D:2 * D], w_qkv[:, 2 * D:3 * D]
    bq, bk, bv = b_qkv[0:D], b_qkv[D:2 * D], b_qkv[2 * D:3 * D]
    in_maps = []
    for c in range(NCORES):
        b = c // GROUPS
        h0 = (c % GROUPS) * HLOC
        cols = slice(h0 * HD, (h0 + HLOC) * HD)
        w_raw = np.concatenate([wq[:, cols], wk[:, cols], wv[:, cols]], axis=1)
        b_c = np.concatenate([bq[cols], bk[cols], bv[cols]]) + ln_bias @ w_raw
        w_c = ln_scale[:, None] * w_raw
        # fold qk-LN per-head mean (blockcenter q/k head column blocks), then
        # fold input-LN mean (colcenter all columns)
        qk_blk = w_c[:, 0:NQK].reshape(D, 8, HD)
        w_c[:, 0:NQK] = (qk_blk - qk_blk.mean(axis=2, keepdims=True)).reshape(D, NQK)
        w_c = w_c - w_c.mean(axis=0, keepdims=True)
        wout_c = w_out[cols, :]
        bout_c = b_out if (c % GROUPS) == 0 else np.zeros_like(b_out)
        xb = np.asarray(x[b], dtype=np.float32)
        # xT packed [p][t][k][tl] = x[128t+tl, 128k+p]
        x4 = xb.reshape(ST, P, KC, P)                       # [t, tl, k, p]
        xT_pack = np.ascontiguousarray(
            x4.transpose(3, 0, 2, 1).reshape(P, ST * KC * P))
        in_maps.append({
            "x": np.ascontiguousarray(xb).astype(F16NP),
            "xT": xT_pack.astype(F16NP),
            "wqkv": _tile_rows(w_c).astype(F16NP),
            "wout": _tile_rows(wout_c).astype(F16NP),
            "bout": bout_c.reshape(1, -1).astype(F16NP),
            "_bqkv_check": b_c,
            **tabs_tiled,
        })
    return in_maps


def kernel(x, w_qkv, b_qkv, w_out, b_out, ln_scale, ln_bias, q_scale, k_scale):
    args = [np.asarray(a, dtype=np.float32) for a in
            (x, w_qkv, b_qkv, w_out, b_out, ln_scale, ln_bias, q_scale, k_scale)]
    in_maps = make_in_maps(*args)
    use_bias_qkv = any(np.any(m.pop("_bqkv_check")) for m in in_maps)
    use_bias_out = any(np.any(m["bout"]) for m in in_maps)
    nc = _get_program(use_bias_qkv, use_bias_out)

    res = run_bass_kernel_spmd(nc, in_maps, core_ids=list(range(NCORES)))

    out = np.zeros((B, S, D), dtype=np.float32)
    for c in range(NCORES):
        out[c // GROUPS] += res.results[c]["out"]
    return out


# revision 83
# speedup vs baseline: 1.3247x; 1.3247x over previous
"""Trainium2 Bass kernel for nn_Attention_146028888114.

Full attention block: LN -> QKV -> per-head QK-LN -> RoPE -> SDPA -> out-proj.
B=2, S=2048, D=1024, H=16, HD=64 (fp16 compute, f32 PSUM accumulation).

Sharding: DP over batch (2 groups of 4 cores) x TP over heads (4 heads/core).
Each core computes a partial out-projection (its 4 heads' contribution); the
host sums the 4 partials per batch (the unshard/reduce step).

Structure (v2):
- Host supplies x twice in fp16: token-major (LN stats) and pre-transposed
  per-tile (matmul lhsT) -- no PE transposes of x.
- Input LN folded into the QKV matmul: qkv = (x - mu) @ W via a rank-1
  correction matmul (lhsT = -mu row, rhs = column-sums of W). The 1/std
  factor cancels in the per-head QK-LN for q/k and is applied to v as a
  per-partition ACT scale.
- Per-head QK-LN stats from one grouped bn_stats (even/odd 6-stat form).
- Phase 2: head-pair outermost; scores -> exp -> attnV software-pipelined so
  ACT (exp) stays saturated; softmax normalize + out-proj for chunk j are
  deferred into chunk j+1's loop.
"""

import sys

sys.path.insert(0, "/opt/trn_rl_repo")

import numpy as np

import concourse.bass as bass
import concourse.tile as tile
from concourse import bacc, mybir
from concourse.alu_op_type import AluOpType
from concourse.bass_utils import run_bass_kernel_spmd
from concourse.masks import make_identity

B, S, D = 2, 2048, 1024
H, HD = 16, 64
EPS = 1e-6
ROPE_BASE = 10000.0

NCORES = 8
GROUPS = 4            # cores per batch group (TP degree)
HLOC = H // GROUPS    # heads per core
P = 128
ST = S // P           # 16 s-tiles
KC = D // P           # 8 k-chunks of the QKV contraction
NQKV = 3 * HLOC * HD  # 768 qkv columns per core
NQK = 2 * HLOC * HD   # 512 q+k columns
SQW = 512             # sq chunk width
SQC = S // SQW        # 4 sq chunks
# v cols + ones col at 64 (denominator trick); padded to a 32 multiple:
# matmul silently zeroes output rows past M=64 when M isn't 32-aligned.
VW = 96

DT = mybir.dt
F32 = DT.float32
F16 = DT.float16
AF = mybir.ActivationFunctionType
F16NP = np.float16


def build_program(use_bias_qkv: bool, use_bias_out: bool, stop: int = 5,
                  debug_dump: bool = False, repeat: int = 1):
    if use_bias_qkv:
        raise NotImplementedError("non-zero qkv/ln bias not supported by v2 kernel")
    nc = bacc.Bacc("TRN2", target_bir_lowering=False, debug=False, num_devices=NCORES)

    x_d = nc.dram_tensor("x", [S, D], F16, kind="ExternalInput")
    xT_d = nc.dram_tensor("xT", [P, ST * KC * P], F16, kind="ExternalInput")
    wqkv_d = nc.dram_tensor("wqkv", [P, KC * NQKV], F16, kind="ExternalInput")
    wout_d = nc.dram_tensor("wout", [P, 2 * D], F16, kind="ExternalInput")
    bout_d = nc.dram_tensor("bout", [1, D], F16, kind="ExternalInput")
    tab_d = {nm: nc.dram_tensor(nm, [P, ST * HD], F16, kind="ExternalInput")
             for nm in ("cq", "sq", "ck", "sk")}
    out_d = nc.dram_tensor("out", [S, D], F32, kind="ExternalOutput")
    rec_d = nc.dram_tensor("recscratch", [2 * SQC, 2 * SQW], F16)
    if debug_dump:
        dbg_qT = nc.dram_tensor("dbg_qT", [P, 2 * S], F16, kind="ExternalOutput")
        dbg_kT = nc.dram_tensor("dbg_kT", [P, 2 * S], F16, kind="ExternalOutput")
        dbg_v = nc.dram_tensor("dbg_v", [P, ST * HLOC * VW], F16, kind="ExternalOutput")
        dbg_an = nc.dram_tensor("dbg_an", [P, 2 * S], F16, kind="ExternalOutput")
        dbg_qkn = nc.dram_tensor("dbg_qkn", [S, 8 * HD], F16, kind="ExternalOutput")

    with tile.TileContext(nc) as tc:
        with tc.tile_pool(name="const", bufs=1) as cpool, \
             tc.tile_pool(name="data", bufs=1) as dpool:
            # --- constants ---
            ident = cpool.tile([P, P], F32)
            make_identity(nc, ident[:])
            ident_h = cpool.tile([P, P], F16)
            nc.vector.tensor_copy(ident_h[:], ident[:])

            ones1h = cpool.tile([1, P], F16)
            nc.vector.memset(ones1h[:], 1.0)

            eps_t = cpool.tile([P, 1], F32)
            nc.vector.memset(eps_t[:], EPS)
            eps64_t = cpool.tile([P, 1], F32)
            nc.vector.memset(eps64_t[:], float(HD) * EPS)
            eps9 = cpool.tile([P, 9], F32)
            nc.vector.memset(eps9[:], float(HD) * EPS)
            nc.vector.memset(eps9[:, 0:1], EPS)

            # --- persistent data tiles ---
            qT = dpool.tile([P, 2, S], F16, tag="qT")   # [pair-features, hp, s]
            kT = dpool.tile([P, 2, S], F16, tag="kT")
            v_all = dpool.tile([P, ST, HLOC, VW], F16, tag="v")
            attnN = dpool.tile([P, 2, S], F16, tag="attnN")

            # --- weights / tables: host supplies fp16 pre-tiled, direct DMA ---
            w_r = cpool.tile([P, KC, NQKV], F16)
            nc.sync.dma_start(w_r[:].rearrange("p a b -> p (a b)"), wqkv_d[:])
            wout_r = cpool.tile([P, 2, D], F16)
            nc.sync.dma_start(wout_r[:].rearrange("p a b -> p (a b)"), wout_d[:])
            if use_bias_out:
                bo_r = cpool.tile([1, D], F16)
                nc.sync.dma_start(bo_r[:], bout_d[:])
            tabs = {}
            for nm, dram in tab_d.items():
                tt = cpool.tile([P, ST, HD], F16, tag=f"tab_{nm}")
                nc.sync.dma_start(tt[:].rearrange("p a b -> p (a b)"), dram[:])
                tabs[nm] = tt

            # ones columns of v (denominator trick): one ACT op
            onescol_f = cpool.tile([P, 1], F32)
            nc.vector.memset(onescol_f[:], 1.0)
            nc.scalar.activation(
                v_all[:, :, :, HD:VW],
                onescol_f[:, :, None, None].to_broadcast((P, ST, HLOC, VW - HD)),
                AF.Copy,
            )

            for _rep in range(repeat):
              # ------------- Phase 1: LN-folded QKV + QK-LN + RoPE ------------
              if stop >= 2:
                with tc.tile_pool(name="xp", bufs=4) as xp, \
                   tc.tile_pool(name="xtp", bufs=4) as xtp, \
                   tc.tile_pool(name="ph1", bufs=4) as ph1, \
                   tc.tile_pool(name="ph1s", bufs=8) as ph1s, \
                   tc.tile_pool(name="tq_ps", bufs=2, space="PSUM") as tqp, \
                   tc.tile_pool(name="qkv_ps", bufs=3, space="PSUM") as qkvp:
                  rope_pending = []  # deferred rope-transposes (one-tile lag)
                  for t in range(ST):
                      x_t = xp.tile([P, 2, 512], F16, tag="x")
                      nc.sync.dma_start(
                          x_t[:].rearrange("p a b -> p (a b)"),
                          x_d[t * P:(t + 1) * P, :])
                      xT_t = xtp.tile([P, KC, P], F16, tag="xT")
                      nc.sync.dma_start(
                          xT_t[:].rearrange("p a b -> p (a b)"),
                          xT_d[:, t * KC * P:(t + 1) * KC * P])

                      # input LN stats (grouped bn_stats + aggregate)
                      st1 = ph1s.tile([P, 2, 6], F32, tag="st1")
                      nc.vector.bn_stats(st1[:, 0, :], x_t[:, 0, :])
                      nc.vector.bn_stats(st1[:, 1, :], x_t[:, 1, :])
                      mv = ph1s.tile([P, 2], F32, tag="mv")
                      nc.vector.bn_aggr(mv[:], st1[:])
                      rstd = ph1s.tile([P, 1], F32, tag="rstd")
                      nc.scalar.activation(rstd[:], mv[:, 1:2], AF.Sqrt, bias=eps_t[:])
                      nc.vector.reciprocal(rstd[:], rstd[:])

                      # QKV projection (both LN means pre-folded into weights)
                      qkv_ps = qkvp.tile([P, NQKV], F32, tag="qkv")
                      nsl = [(0, 512), (512, NQKV)]
                      for k in range(KC):
                          for lo, hi in nsl:
                              nc.tensor.matmul(
                                  qkv_ps[:, lo:hi], lhsT=xT_t[:, k, :],
                                  rhs=w_r[:, k, lo:hi],
                                  start=(k == 0), stop=(k == KC - 1),
                              )

                      # previous tile's rope transposes after this tile's QKV:
                      # by then their (elementwise-produced) inputs are ready,
                      # so PE never waits on the LN/rope chain
                      for fn in rope_pending:
                          fn()
                      rope_pending = []

                      # v with 1/std scale (per-partition ACT scale)
                      nc.scalar.activation(
                          v_all[:, t, :, 0:HD],
                          qkv_ps[:, NQK:NQKV].rearrange("p (h d) -> p h d", h=HLOC),
                          AF.Copy, scale=rstd[:],
                      )

                      # per-head QK LN: qk is already head-centered (mean folded
                      # into weights), so only the variance is needed:
                      # rstd = 1/sqrt(sumsq/64 + eps) = 8/sqrt(sumsq + 64*eps)
                      qk_g = qkv_ps[:, 0:NQK].rearrange("p (g d) -> p g d", g=8)
                      sq = ph1.tile([P, 8, HD], F16, tag="sq")
                      nc.scalar.square(
                          sq[:].rearrange("p g d -> p (g d)"), qkv_ps[:, 0:NQK])
                      sumsq = ph1s.tile([P, 8], F32, tag="sumsq")
                      nc.vector.reduce_sum(sumsq[:], sq[:], axis=mybir.AxisListType.X)
                      s8 = ph1s.tile([P, 8], F32, tag="s8")
                      nc.scalar.activation(s8[:], sumsq[:], AF.Sqrt, bias=eps64_t[:])
                      r8 = ph1s.tile([P, 8], F32, tag="r8")
                      nc.vector.reciprocal(r8[:], s8[:])
                      r88 = ph1s.tile([P, 8], F32, tag="r88")
                      nc.vector.tensor_scalar_mul(r88[:], r8[:], 8.0)
                      qkn = ph1.tile([P, 8, HD], F16, tag="qkn")
                      nc.vector.tensor_tensor(
                          qkn[:], qk_g, r88[:, :, None].to_broadcast((P, 8, HD)),
                          op=AluOpType.mult)
                      if debug_dump:
                          nc.sync.dma_start(
                              dbg_qkn[t * P:(t + 1) * P, :],
                              qkn[:].rearrange("p a b -> p (a b)"))

                      # RoPE (q on DVE; k split DVE/Pool) + PE pair-transposes
                      for gsl, ctab, stab, dstT, eng2, eng34 in (
                              (slice(0, HLOC), "cq", "sq", qT, nc.vector, nc.vector),
                              (slice(HLOC, 8), "ck", "sk", kT, nc.gpsimd, nc.gpsimd)):
                          n_t = qkn[:, gsl, :]
                          ct = tabs[ctab][:, t, None, :].to_broadcast((P, HLOC, HD))
                          s_lo = tabs[stab][:, t, None, 0:32].to_broadcast((P, HLOC, 32))
                          s_hi = tabs[stab][:, t, None, 32:64].to_broadcast((P, HLOC, 32))
                          r2 = ph1.tile([P, HLOC, HD], F16, tag="r2")
                          eng2.tensor_tensor(
                              r2[:, :, 0:32], n_t[:, :, 32:64], s_lo, op=AluOpType.mult)
                          eng2.tensor_tensor(
                              r2[:, :, 32:64], n_t[:, :, 0:32], s_hi, op=AluOpType.mult)
                          r3 = ph1.tile([P, HLOC, HD], F16, tag="r3")
                          eng34.tensor_tensor(r3[:], n_t[:], ct, op=AluOpType.mult)
                          r4 = ph1.tile([P, HLOC, HD], F16, tag="r4")
                          eng34.tensor_tensor(r4[:], r3[:], r2[:], op=AluOpType.add)

                          # tq = T(r4 pair); deferred one tile so PE never
                          # waits on the LN/rope chain of the current tile
                          def make_ropet(r4=r4, dstT=dstT, t=t):
                              def _ropet():
                                  tq = tqp.tile([P, 2, P], F16, tag="tq",
                                                name="tq")
                                  for hp in range(2):
                                      nc.tensor.matmul(
                                          tq[:, hp, :],
                                          lhsT=r4[:, 2 * hp:2 * hp + 2, :].rearrange("p h d -> p (h d)"),
                                          rhs=ident_h[:],
                                          is_transpose=True, start=True, stop=True,
                                      )
                                  nc.scalar.activation(
                                      dstT[:, :, t * P:(t + 1) * P], tq[:], AF.Copy)
                              return _ropet
                          rope_pending.append(make_ropet())
                  for fn in rope_pending:
                      fn()
                  rope_pending = []

              if debug_dump:
                  with tc.tile_pool(name="dbgp", bufs=2) as dbgp:
                      for (dst, srcT) in ((dbg_qT, qT), (dbg_kT, kT)):
                          nc.sync.dma_start(dst[:], srcT[:].rearrange("p a b -> p (a b)"))
                      nc.sync.dma_start(
                          dbg_v[:], v_all[:].rearrange("p a b c -> p (a b c)"))

              # ------------- Phase 2+3: attention + out-projection ------------
              if stop >= 3:
                with tc.tile_pool(name="out_ps", bufs=2, space="PSUM") as outp, \
                   tc.tile_pool(name="attn_ps", bufs=1, space="PSUM") as atp, \
                   tc.tile_pool(name="st_ps", bufs=2, space="PSUM") as stp, \
                   tc.tile_pool(name="pt", bufs=8) as ptp, \
                   tc.tile_pool(name="ph3", bufs=2) as ph3, \
                   tc.tile_pool(name="ob", bufs=3) as obp:

                  def make_normalize(hp, j, asb, bc):
                      def _norm():
                          for h2 in range(2):
                              nc.gpsimd.tensor_tensor(
                                  attnN[h2 * HD:(h2 + 1) * HD, hp,
                                        j * SQW:(j + 1) * SQW],
                                  asb[:, h2, :], bc[:, h2, :], op=AluOpType.mult)
                      return _norm

                  # out-proj of one 128-row x 512-col block of chunk j, split
                  # into two single-matmul pieces so the interleave never puts
                  # more than ~213ns of extra PE work into one i-iteration
                  def make_outproj(j, m, lo_i):
                      row = j * SQW + m * P
                      lo = lo_i * 512
                      state = {}

                      def _op1():
                          o_ps = outp.tile([P, 512], F32, tag="out", name="o_ps")
                          state["o_ps"] = o_ps
                          first = True
                          if use_bias_out:
                              nc.tensor.matmul(
                                  o_ps[:], lhsT=ones1h[:], rhs=bo_r[:, lo:lo + 512],
                                  start=True, stop=False)
                              first = False
                          nc.tensor.matmul(
                              o_ps[:], lhsT=attnN[:, 0, row:row + P],
                              rhs=wout_r[:, 0, lo:lo + 512],
                              start=first, stop=False)

                      def _op2():
                          o_ps = state["o_ps"]
                          nc.tensor.matmul(
                              o_ps[:], lhsT=attnN[:, 1, row:row + P],
                              rhs=wout_r[:, 1, lo:lo + 512],
                              start=False, stop=True)
                          ob = obp.tile([P, 512], F32, tag="ob", name="ob")
                          if j == SQC - 1:
                              # final chunk drains with ACT idle; DVE is busy
                              # with this chunk's recips/copies
                              nc.scalar.copy(ob[:], o_ps[:])
                          else:
                              nc.vector.tensor_copy(ob[:], o_ps[:])
                          nc.sync.dma_start(out_d[row:row + P, lo:lo + 512], ob[:])

                      return [_op1, _op2]

                  pending = []
                  chunks = [(hp, j) for hp in range(2) for j in range(SQC)]
                  accs = {}
                  lag_q = []   # (ci, i, pT) whose attnV is deferred 2 slots
                  LAG = 2

                  def emit_attnv(ci, i, pT):
                      hp, j = chunks[ci]
                      for h2 in range(2):
                          nc.tensor.matmul(
                              accs[ci][h2][:],
                              lhsT=v_all[:, i, 2 * hp + h2, :],
                              rhs=pT[:, h2, :],
                              start=(i == 0), stop=(i == ST - 1))

                  def emit_tail(ci):
                      hp, j = chunks[ci]
                      acc = accs.pop(ci)
                      last = (ci == len(chunks) - 1)
                      asb = ph3.tile([HD, 2, SQW], F16, tag="asb", name="asb")
                      rec = ph3.tile([1, 2, SQW], F16, tag="rec", name="rec")
                      with nc.allow_low_precision(reason="fp16 softmax recip"):
                          for h2 in range(2):
                              nc.vector.reciprocal(
                                  rec[:, h2, :], acc[h2][HD:HD + 1, :])
                              nc.vector.tensor_copy(asb[:, h2, :], acc[h2][0:HD, :])
                      ridx = hp * SQC + j
                      nc.sync.dma_start(
                          rec_d[ridx:ridx + 1, :],
                          rec[:].rearrange("p a b -> p (a b)"))
                      bc = ph3.tile([HD, 2, SQW], F16, tag="bc", name="bc")
                      nc.sync.dma_start(
                          bc[:].rearrange("p a b -> p (a b)"),
                          rec_d[ridx:ridx + 1, :].to_broadcast((HD, 2 * SQW)))
                      if last and stop >= 5:
                          # drain: column-split normalize so each out-proj row
                          # block starts as soon as its 128 query columns are
                          # normalized
                          for m in range(SQW // P):
                              cl = slice(m * P, (m + 1) * P)
                              for h2 in range(2):
                                  nc.gpsimd.tensor_tensor(
                                      attnN[h2 * HD:(h2 + 1) * HD, hp,
                                            j * SQW + m * P:j * SQW + (m + 1) * P],
                                      asb[:, h2, cl], bc[:, h2, cl],
                                      op=AluOpType.mult)
                              for lo_i in range(2):
                                  for fn in make_outproj(j, m, lo_i):
                                      fn()
                      else:
                          pending.append(make_normalize(hp, j, asb, bc))
                          if hp == 1 and stop >= 5:
                              for m in range(SQW // P):
                                  for lo_i in range(2):
                                      pending.extend(make_outproj(j, m, lo_i))

                  for ci, (hp, j) in enumerate(chunks):
                      accs[ci] = [atp.tile([VW, SQW], F32, tag=f"acc{h2}",
                                           name=f"acc{h2}")
                                  for h2 in range(2)]
                      for i in range(ST):
                          sT = stp.tile([P, 2, SQW], F32, tag="st")
                          for h2 in range(2):
                              fl = h2 * HD
                              nc.tensor.matmul(
                                  sT[:, h2, :],
                                  lhsT=kT[fl:fl + HD, hp, i * P:(i + 1) * P],
                                  rhs=qT[fl:fl + HD, hp, j * SQW:(j + 1) * SQW],
                                  start=True, stop=True)
                          pT = ptp.tile([P, 2, SQW], F16, tag="pt")
                          nc.scalar.activation(pT[:], sT[:], AF.Exp, scale=0.125)
                          if pending and (ci > 0 or i >= 1):
                              pending.pop(0)()
                              if len(pending) > 16 - i:
                                  pending.pop(0)()
                          lag_q.append((ci, i, pT))
                          if len(lag_q) > LAG:
                              aci, ai, apT = lag_q.pop(0)
                              emit_attnv(aci, ai, apT)
                              if ai == ST - 1:
                                  emit_tail(aci)
                  while lag_q:
                      aci, ai, apT = lag_q.pop(0)
                      emit_attnv(aci, ai, apT)
                      if ai == ST - 1:
                          for fn in pending:
                              fn()
                          pending = []
                          emit_tail(aci)

              if debug_dump:
                  with tc.tile_pool(name="dbgp2", bufs=1) as dbgp2:
                      nc.sync.dma_start(
                          dbg_an[:], attnN[:].rearrange("p a b -> p (a b)"))

    nc.compile()
    return nc


_PROGRAM_CACHE = {}


def _get_program(use_bias_qkv, use_bias_out):
    key = (use_bias_qkv, use_bias_out)
    if key not in _PROGRAM_CACHE:
        _PROGRAM_CACHE[key] = build_program(*key)
    return _PROGRAM_CACHE[key]


def _rope_tables(q_scale, k_scale):
    inv_freq = 1.0 / ROPE_BASE ** (np.arange(0, HD, 2, dtype=np.float32) / HD)
    t = np.arange(S, dtype=np.float32)
    freqs = np.einsum("i,j->ij", t, inv_freq)
    emb = np.concatenate((freqs, freqs), axis=-1)          # [S, HD]
    cos = np.cos(emb).astype(np.float32)
    sin = np.sin(emb).astype(np.float32)
    sgnsin = sin.copy()
    sgnsin[:, 0:HD // 2] *= -1.0
    swap = lambda v: np.concatenate((v[HD // 2:], v[:HD // 2]))
    tabs = {}
    for nm, sc in (("q", q_scale), ("k", k_scale)):
        tabs["c" + nm] = np.ascontiguousarray(cos * sc[None, :])
        tabs["s" + nm] = np.ascontiguousarray(sgnsin * swap(sc)[None, :])
    return tabs


def _tile_rows(a):
    """[S-like rows, W] -> [P, (rows/P) * W] partition-tiled layout."""
    r, w = a.shape
    return np.ascontiguousarray(
        a.reshape(r // P, P, w).transpose(1, 0, 2).reshape(P, (r // P) * w))


def make_in_maps(x, w_qkv, b_qkv, w_out, b_out, ln_scale, ln_bias, q_scale, k_scale):
    tabs = _rope_tables(q_scale, k_scale)
    tabs_tiled = {nm: _tile_rows(v).astype(F16NP) for nm, v in tabs.items()}
    wq, wk, wv = w_qkv[:, 0:D], w_qkv[:, D:2 * D], w_qkv[:, 2 * D:3 * D]
    bq, bk, bv = b_qkv[0:D], b_qkv[D:2 * D], b_qkv[2 * D:3 * D]
    in_maps = []
    for c in range(NCORES):
        b = c // GROUPS
        h0 = (c % GROUPS) * HLOC
        cols = slice(h0 * HD, (h0 + HLOC) * HD)
        w_raw = np.concatenate([wq[:, cols], wk[:, cols], wv[:, cols]], axis=1)
        b_c = np.concatenate([bq[cols], bk[cols], bv[cols]]) + ln_bias @ w_raw
        w_c = ln_scale[:, None] * w_raw
        # fold qk-LN per-head mean (blockcenter q/k head column blocks), then
        # fold input-LN mean (colcenter all columns)
        qk_blk = w_c[:, 0:NQK].reshape(D, 8, HD)
        w_c[:, 0:NQK] = (qk_blk - qk_blk.mean(axis=2, keepdims=True)).reshape(D, NQK)
        w_c = w_c - w_c.mean(axis=0, keepdims=True)
        wout_c = w_out[cols, :]
        bout_c = b_out if (c % GROUPS) == 0 else np.zeros_like(b_out)
        xb = np.asarray(x[b], dtype=np.float32)
        # xT packed [p][t][k][tl] = x[128t+tl, 128k+p]
        x4 = xb.reshape(ST, P, KC, P)                       # [t, tl, k, p]
        xT_pack = np.ascontiguousarray(
            x4.transpose(3, 0, 2, 1).reshape(P, ST * KC * P))
        in_maps.append({
            "x": np.ascontiguousarray(xb).astype(F16NP),
            "xT": xT_pack.astype(F16NP),
            "wqkv": _tile_rows(w_c).astype(F16NP),
            "wout": _tile_rows(wout_c).astype(F16NP),
            "bout": bout_c.reshape(1, -1).astype(F16NP),
            "_bqkv_check": b_c,
            **tabs_tiled,
        })
    return in_maps


def kernel(x, w_qkv, b_qkv, w_out, b_out, ln_scale, ln_bias, q_scale, k_scale):
    args = [np.asarray(a, dtype=np.float32) for a in
            (x, w_qkv, b_qkv, w_out, b_out, ln_scale, ln_bias, q_scale, k_scale)]
    in_maps = make_in_maps(*args)
    use_bias_qkv = any(np.any(m.pop("_bqkv_check")) for m in in_maps)
    use_bias_out = any(np.any(m["bout"]) for m in in_maps)
    nc = _get_program(use_bias_qkv, use_bias_out)

    res = run_bass_kernel_spmd(nc, in_maps, core_ids=list(range(NCORES)))

    out = np.zeros((B, S, D), dtype=np.float32)
    for c in range(NCORES):
        out[c // GROUPS] += res.results[c]["out"]
    return out
